# revision 22
# baseline (speedup 1.0000x reference)
"""DepthDC fused kernel for 8 Trainium2 NeuronCores (fp16 datapath).

Reference computation (N=2, C=64, H=W=256, d=2):
  patches[n,c,k,h,w] = xpad[n,c,h+ki*d, w+kj*d]   (k=3*ki+kj, pad d)
  out1 = sum_k patches * y.reshape(N,C,9,H,W)
  out  = leaky_relu(conv3x3(out1, fuse_w) + fuse_b, 0.2)

Sharding: 8 cores = batch(2) x H-quarters(4). Each core produces a
[64, 64, 256] output slab. Host slices overlapping (haloed, zero-padded)
input slabs per core and converts them to fp16, so no device collectives
are needed and HBM traffic is halved vs fp32 (y dominates at ~19 MB/core).

Per-core layout: the 64 output rows split into two 32-row halves mapped
to SBUF partition halves (partition = c + 64*s). All engines see uniform
[128, F] tiles. Host pre-packs every DRAM tensor so each DMA is fully
contiguous per partition.

Engines:
  - DVE: 9 elementwise products per chunk (fp16, 2x mode) and, for
    DVE-reduction chunks, a tree of 8 adds
  - PE:  for PE-reduction chunks, k-sum via identity matmul in PSUM;
    3x3 dense conv as 9 accumulating fp16 matmuls over C=64
  - ACT: all out1 (o1_sb) writes (PSUM or SBUF copies) plus the fused
    bias + leaky-relu on the conv PSUM output
Work is streamed over 4-row h-chunks with double-buffered y DMA.
"""

import sys

sys.path.insert(0, "/opt/trn_rl_repo")

import numpy as np

import concourse.bass as bass
import concourse.mybir as mybir
import concourse.tile as tile
from concourse import bacc
from concourse.bass_utils import run_bass_kernel_spmd

F16 = mybir.dt.float16
F32 = mybir.dt.float32
AF = mybir.ActivationFunctionType

N, C, H, W = 2, 64, 256, 256
D = 2  # dilation == pad
NEG_SLOPE = 0.2
NCORES = 8
HB = 64          # output rows per core
HH = 32          # output rows per half
Q = HH + 2       # out1 rows per half (34)
XR = Q + 4       # x rows per half block (38)
XW = W + 2 * D   # padded x width (260)
OW = W + 4       # padded out1 width (260; data at cols 2..258)
RC = 4           # rows per chunk
NRED = 9         # reduce chunks per half: 8 x 4 rows + 1 x 2 rows
# per-chunk engine for the k-reduction add tree: Pool (gpsimd) or DVE.
# PE is conv-only — on this device PE is utilization-throttled to
# ~1.2 GHz, so putting reduction matmuls on it made the PE queue the
# bottleneck. The add tree splits roughly 50/50 between DVE and Pool.
POOL_CHUNKS = (0, 2, 4, 6)


def _build_program():
    nc = bacc.Bacc("TRN2", target_bir_lowering=False, debug=False,
                   num_devices=NCORES)

    xq_d = nc.dram_tensor("xq", [128, XR, XW], F16, kind="ExternalInput").ap()
    yq_d = nc.dram_tensor("yq", [128, Q, 9, W], F16,
                          kind="ExternalInput").ap()
    wt_d = nc.dram_tensor("wt", [128, 9, 128], F16, kind="ExternalInput").ap()
    b_d = nc.dram_tensor("bias", [128, 1], F32, kind="ExternalInput").ap()
    out_d = nc.dram_tensor("out", [128, HH, W], F16, kind="ExternalOutput").ap()

    with tile.TileContext(nc) as tc:
        from contextlib import ExitStack
        with ExitStack() as ctx:
            const = ctx.enter_context(tc.tile_pool(name="const", bufs=1))
            y_pool = ctx.enter_context(tc.tile_pool(name="y_pool", bufs=2))
            p_pool = ctx.enter_context(tc.tile_pool(name="p_pool", bufs=2))
            s_pool = ctx.enter_context(tc.tile_pool(name="s_pool", bufs=2))
            o_pool = ctx.enter_context(tc.tile_pool(name="o_pool", bufs=2))
            v_pool = ctx.enter_context(tc.tile_pool(name="v_pool", bufs=2))
            ps2_pool = ctx.enter_context(
                tc.tile_pool(name="ps2_pool", bufs=4, space="PSUM"))

            # constants / whole-slab x / whole-slab out1
            w_sb = const.tile([128, 9, 128], F16, name="w_sb")
            nc.sync.dma_start(w_sb[:], wt_d)
            b_sb = const.tile([128, 1], F32, name="b_sb")
            nc.sync.dma_start(b_sb[:], b_d)
            # x arrives in two pieces so chunk-0 products can start after
            # ~3us instead of waiting for the whole 38-row slab. Rows
            # [0:16) cover the x reads of chunks 0-2; the rest lands well
            # before chunk 3 needs it (its DMA is issued after chunk 1's
            # y DMA below).
            x_sb = const.tile([128, XR, XW], F16, name="x_sb")
            nc.sync.dma_start(x_sb[:, 0:16], xq_d[:, 0:16])
            o1_sb = const.tile([128, Q, OW], F16, name="o1_sb")
            # zero the conv W-padding columns once (ACT, scale=0 writes 0)
            nc.scalar.activation(o1_sb[:, :, 1:2], o1_sb[:, :, 1:2],
                                 AF.Copy, scale=0.0)
            nc.scalar.activation(o1_sb[:, :, OW - 2:OW - 1],
                                 o1_sb[:, :, OW - 2:OW - 1], AF.Copy,
                                 scale=0.0)
            # Wait-merge scratch: one cheap copy per input DMA converts
            # DMA-completion semaphores into engine program order, so
            # compute instructions never need more than 1-2 foreign wait
            # sems (the TT-struct wait-slot limit in walrus codegen is
            # tight). DVE covers x/w/id/b; ACT additionally covers b and
            # w (its Lrelu reads b_sb, conv matmuls read w_sb after
            # waiting on ACT's o1 writes).
            scr = const.tile([128, 8], F16, name="scr")
            nc.vector.tensor_copy(scr[:, 0:1], x_sb[:, 0, 0:1])
            nc.vector.tensor_copy(scr[:, 2:3], w_sb[:, 0, 0:1])
            scr2 = const.tile([128, 2], F32, name="scr2")
            nc.scalar.activation(scr2[:, 0:1], b_sb[:, 0:1], AF.Copy)
            nc.scalar.activation(scr2[:, 1:2], w_sb[:, 0, 0:1], AF.Copy)

            def reduce_chunk(c):
                q0 = RC * c
                rc = min(RC, Q - q0)
                y_t = y_pool.tile([128, RC, 9, W], F16, name="y_t", tag="y_t")
                nc.sync.dma_start(y_t[:, 0:rc], yq_d[:, q0:q0 + rc])
                if c == 1:
                    nc.sync.dma_start(x_sb[:, 16:XR], xq_d[:, 16:XR])
                nc.vector.tensor_copy(scr[:, 5:6], y_t[:, 0, 0, 0:1])
                p_t = p_pool.tile([128, 9, RC, W], F16, name="p_t", tag="p_t")
                for k in range(9):
                    ki, kj = divmod(k, 3)
                    x_view = x_sb[:, q0 + 2 * ki: q0 + 2 * ki + rc,
                                  2 * kj: 2 * kj + W]
                    nc.vector.tensor_mul(p_t[:, k, 0:rc], x_view,
                                         y_t[:, 0:rc, k])
                eng = (nc.gpsimd if c in POOL_CHUNKS else nc.vector)
                s_t = s_pool.tile([128, 7, RC, W], F16, name="s_t",
                                  tag="s_t")
                with nc.allow_low_precision("fp16 k-sum; tol 2e-2"):
                    a = eng.tensor_add
                    a(s_t[:, 0, 0:rc], p_t[:, 0, 0:rc], p_t[:, 1, 0:rc])
                    a(s_t[:, 1, 0:rc], p_t[:, 2, 0:rc], p_t[:, 3, 0:rc])
                    a(s_t[:, 2, 0:rc], p_t[:, 4, 0:rc], p_t[:, 5, 0:rc])
                    a(s_t[:, 3, 0:rc], p_t[:, 6, 0:rc], p_t[:, 7, 0:rc])
                    a(s_t[:, 4, 0:rc], s_t[:, 0, 0:rc], s_t[:, 1, 0:rc])
                    a(s_t[:, 5, 0:rc], s_t[:, 2, 0:rc], s_t[:, 3, 0:rc])
                    a(s_t[:, 6, 0:rc], s_t[:, 4, 0:rc], s_t[:, 5, 0:rc])
                    a(s_t[:, 0, 0:rc], s_t[:, 6, 0:rc], p_t[:, 8, 0:rc])
                # all o1 writes go through ACT so conv matmuls wait on
                # a single engine (plus the w DMA, covered at startup)
                nc.scalar.copy(o1_sb[:, q0:q0 + rc, 2:W + 2],
                               s_t[:, 0, 0:rc])
                if c == 2:
                    # late wait-merge for the second x piece: DVE is past
                    # chunk-2 work, the DMA finished long ago, so this
                    # costs nothing and keeps chunk-3+ muls at <=2 waits
                    nc.vector.tensor_copy(scr[:, 1:2], x_sb[:, XR - 1, 0:1])

            def conv_chunk(j):
                m0 = RC * j
                ps2 = ps2_pool.tile([128, RC, W], F32, name="ps2", tag="ps2")
                for t in range(9):
                    i3, j3 = divmod(t, 3)
                    for r0 in (0, 2):
                        nc.tensor.matmul(
                            ps2[:, r0:r0 + 2], lhsT=w_sb[:, t],
                            rhs=o1_sb[:, m0 + i3 + r0: m0 + i3 + r0 + 2,
                                      j3 + 1: j3 + 1 + W],
                            start=(t == 0), stop=(t == 8))
                # bias on ACT (PSUM read), leaky combine on DVE:
                # out = max(v, 0.2*v), v = ps2 + b
                v_t = v_pool.tile([128, RC, W], F16, name="v_t", tag="v_t")
                nc.scalar.activation(v_t[:], ps2[:], AF.Identity,
                                     bias=b_sb[:, 0:1], scale=1.0)
                o_t = o_pool.tile([128, RC, W], F16, name="o_t", tag="o_t")
                nc.vector.scalar_tensor_tensor(
                    o_t[:], v_t[:], NEG_SLOPE, v_t[:],
                    mybir.AluOpType.mult, mybir.AluOpType.max)
                nc.sync.dma_start(out_d[:, m0:m0 + RC], o_t[:])

            for c in range(NRED):
                reduce_chunk(c)
                if c >= 1:
                    conv_chunk(c - 1)

    nc.compile()
    return nc


_PROGRAM = None


def _get_program():
    global _PROGRAM
    if _PROGRAM is None:
        _PROGRAM = _build_program()
    return _PROGRAM


def make_in_maps(x, y, fuse_w, fuse_b):
    x16 = np.asarray(x).astype(np.float16)
    y16 = np.asarray(y).astype(np.float16)
    fuse_w = np.asarray(fuse_w, dtype=np.float32)
    fuse_b = np.asarray(fuse_b, dtype=np.float32)

    # block-diagonal conv weights: each partition half (h-half of the
    # slab) contracts with its own copy of W_tap in one K=128 matmul
    wt = np.zeros((128, 9, 128), np.float16)
    for t in range(9):
        i, j = divmod(t, 3)
        wtap = fuse_w[:, :, i, j].T.astype(np.float16)  # [c_in, c_out]
        wt[0:64, t, 0:64] = wtap
        wt[64:128, t, 64:128] = wtap
    bias = np.concatenate([fuse_b, fuse_b]).astype(np.float32)[:, None]

    in_maps = []
    for core in range(NCORES):
        n, hb = divmod(core, 4)
        h0 = hb * HB
        y4 = y16[n].reshape(C, 9, H, W)

        xq = np.zeros((128, XR, XW), np.float16)
        yq = np.zeros((128, Q, 9, W), np.float16)
        for s in (0, 1):
            hs = h0 + HH * s
            # x rows hs-3 .. hs+35 into xq rows 0..38, cols 2..258
            r0, r1 = hs - 3, hs + XR - 3
            c0, c1 = max(r0, 0), min(r1, H)
            xq[64 * s:64 * s + 64, c0 - r0:c1 - r0, D:D + W] = \
                x16[n, :, c0:c1, :]
            # y rows hs-1 .. hs+33 into yq rows 0..34, transposed to
            # [c, q, k, w]
            r0y, r1y = hs - 1, hs + Q - 1
            c0y, c1y = max(r0y, 0), min(r1y, H)
            yq[64 * s:64 * s + 64, c0y - r0y:c1y - r0y] = \
                y4[:, :, c0y:c1y, :].transpose(0, 2, 1, 3)
        in_maps.append({"xq": xq, "yq": yq, "wt": wt, "bias": bias})
    return in_maps


def run(x, y, fuse_w, fuse_b, trace=False, **kw):
    nc = _get_program()
    in_maps = make_in_maps(x, y, fuse_w, fuse_b)
    res = run_bass_kernel_spmd(nc, in_maps, list(range(NCORES)),
                               trace=trace, **kw)
    out = np.empty((N, C, H, W), np.float32)
    for core in range(NCORES):
        n, hb = divmod(core, 4)
        h0 = hb * HB
        o = res.results[core]["out"]
        for s in (0, 1):
            out[n, :, h0 + HH * s:h0 + HH * (s + 1), :] = \
                o[64 * s:64 * s + 64].astype(np.float32)
    return out, res


def kernel(x, y, fuse_w, fuse_b):
    out, _ = run(x, y, fuse_w, fuse_b, trace=False)
    return out


# revision 25
# speedup vs baseline: 1.3298x; 1.3298x over previous
"""DepthDC fused kernel for 8 Trainium2 NeuronCores (fp16 datapath).

Reference computation (N=2, C=64, H=W=256, d=2):
  patches[n,c,k,h,w] = xpad[n,c,h+ki*d, w+kj*d]   (k=3*ki+kj, pad d)
  out1 = sum_k patches * y.reshape(N,C,9,H,W)
  out  = leaky_relu(conv3x3(out1, fuse_w) + fuse_b, 0.2)

Sharding: 8 cores = batch(2) x H-quarters(4). Each core produces a
[64, 64, 256] output slab. Host slices overlapping (haloed, zero-padded)
input slabs per core and converts them to fp16, so no device collectives
are needed and HBM traffic is halved vs fp32 (y dominates at ~19 MB/core).

Per-core layout: the 64 output rows split into two 32-row halves mapped
to SBUF partition halves (partition = c + 64*s). Host pre-packs every
DRAM tensor so each DMA is fully contiguous per partition.

Engine split (PE is utilization-throttled to ~1.2 GHz on this device and
gpsimd streaming poisons shared SBUF bandwidth, so):
  - DVE: the 9 elementwise products per reduce chunk (fp16 2x mode), the
    k-sum as a 4-op merged add tree (stride-2 slices pair 4 adds into
    one instruction), and the final leaky-relu combine
  - PE:  only the 3x3 dense conv (9 taps x 2 row-pair fp16 matmuls,
    PSUM-accumulated)
  - ACT: all o1 writes (keeps conv matmul waits on one engine) and the
    conv bias add from PSUM
Reduce chunks are 6 rows ([6,6,6,6,6,4] over the 34 out1 rows) to
amortize DVE op overheads; conv chunks are 4 rows (PSUM bank pair),
with leaky-relu and the output DMA batched over conv-chunk pairs.
"""

import sys

sys.path.insert(0, "/opt/trn_rl_repo")

import numpy as np

import concourse.bass as bass
import concourse.mybir as mybir
import concourse.tile as tile
from concourse import bacc
from concourse.bass_utils import run_bass_kernel_spmd

F16 = mybir.dt.float16
F32 = mybir.dt.float32
AF = mybir.ActivationFunctionType

N, C, H, W = 2, 64, 256, 256
D = 2  # dilation == pad
NEG_SLOPE = 0.2
NCORES = 8
HB = 64          # output rows per core
HH = 32          # output rows per half
Q = HH + 2       # out1 rows per half (34)
XR = Q + 4       # x rows per half block (38)
XW = W + 2 * D   # padded x width (260)
OW = W + 4       # padded out1 width (260; data at cols 2..258)
RC = 6           # out1 rows per reduce chunk
RCHUNKS = [(0, 6), (6, 6), (12, 6), (18, 6), (24, 6), (30, 4)]
CC = 4           # output rows per conv chunk
# conv chunks to run after each reduce chunk (conv j needs o1 rows
# [4j, 4j+6), available after reduce chunk c when 4j+6 <= end(c))
CONV_AFTER = {1: (0, 1), 2: (2, 3), 3: (4,), 4: (5, 6), 5: (7,)}


def _build_program():
    nc = bacc.Bacc("TRN2", target_bir_lowering=False, debug=False,
                   num_devices=NCORES)

    xq_d = nc.dram_tensor("xq", [128, XR, XW], F16, kind="ExternalInput").ap()
    yq_d = nc.dram_tensor("yq", [128, Q, 9, W], F16,
                          kind="ExternalInput").ap()
    wt_d = nc.dram_tensor("wt", [128, 9, 128], F16, kind="ExternalInput").ap()
    b_d = nc.dram_tensor("bias", [128, 1], F32, kind="ExternalInput").ap()
    out_d = nc.dram_tensor("out", [128, HH, W], F16, kind="ExternalOutput").ap()

    with tile.TileContext(nc) as tc:
        from contextlib import ExitStack
        with ExitStack() as ctx:
            const = ctx.enter_context(tc.tile_pool(name="const", bufs=1))
            y_pool = ctx.enter_context(tc.tile_pool(name="y_pool", bufs=2))
            p_pool = ctx.enter_context(tc.tile_pool(name="p_pool", bufs=2))
            s_pool = ctx.enter_context(tc.tile_pool(name="s_pool", bufs=2))
            o_pool = ctx.enter_context(tc.tile_pool(name="o_pool", bufs=2))
            v_pool = ctx.enter_context(tc.tile_pool(name="v_pool", bufs=2))
            ps2_pool = ctx.enter_context(
                tc.tile_pool(name="ps2_pool", bufs=4, space="PSUM"))

            w_sb = const.tile([128, 9, 128], F16, name="w_sb")
            nc.sync.dma_start(w_sb[:], wt_d)
            b_sb = const.tile([128, 1], F32, name="b_sb")
            nc.sync.dma_start(b_sb[:], b_d)
            # x arrives in two pieces so chunk-0 products can start early;
            # rows [0:22) cover the x reads of reduce chunks 0-2, the rest
            # is issued after chunk 1's y DMA and lands well before chunk 3.
            x_sb = const.tile([128, XR, XW], F16, name="x_sb")
            nc.sync.dma_start(x_sb[:, 0:22], xq_d[:, 0:22])
            o1_sb = const.tile([128, Q, OW], F16, name="o1_sb")
            # zero the conv W-padding columns once; memset writes without
            # reading (a scale=0 ACT copy reads uninitialized SBUF, and
            # NaN*0 = NaN on a cold device)
            nc.gpsimd.memset(o1_sb[:, :, 1:2], 0.0)
            nc.gpsimd.memset(o1_sb[:, :, OW - 2:OW - 1], 0.0)
            # Wait-merge scratch: one cheap copy per input DMA converts
            # DMA-completion semaphores into engine program order, keeping
            # compute instructions at <=2 foreign wait sems (walrus
            # wait-slot limit). DVE covers x/w; ACT covers b and w (conv
            # matmuls wait on ACT's o1 writes, its Lrelu bias read needs b).
            scr = const.tile([128, 8], F16, name="scr")
            nc.vector.tensor_copy(scr[:, 0:1], x_sb[:, 0, 0:1])
            nc.vector.tensor_copy(scr[:, 2:3], w_sb[:, 0, 0:1])
            scr2 = const.tile([128, 3], F32, name="scr2")
            nc.scalar.activation(scr2[:, 0:1], b_sb[:, 0:1], AF.Copy)
            nc.scalar.activation(scr2[:, 1:2], w_sb[:, 0, 0:1], AF.Copy)
            # fold the Pool memsets into ACT program order so conv matmuls
            # keep a single-engine wait on o1
            nc.scalar.activation(scr2[:, 2:3], o1_sb[:, 0, 1:2], AF.Copy)
            nc.scalar.activation(scr2[:, 2:3], o1_sb[:, 0, OW - 2:OW - 1],
                                 AF.Copy)

            def reduce_chunk(c):
                q0, rc = RCHUNKS[c]
                y_t = y_pool.tile([128, RC, 9, W], F16, name="y_t", tag="y_t")
                nc.sync.dma_start(y_t[:, 0:rc], yq_d[:, q0:q0 + rc])
                if c == 1:
                    nc.sync.dma_start(x_sb[:, 22:XR], xq_d[:, 22:XR])
                nc.vector.tensor_copy(scr[:, 5:6], y_t[:, 0, 0, 0:1])
                p_t = p_pool.tile([128, 9, RC, W], F16, name="p_t", tag="p_t")
                for k in range(9):
                    ki, kj = divmod(k, 3)
                    x_view = x_sb[:, q0 + 2 * ki: q0 + 2 * ki + rc,
                                  2 * kj: 2 * kj + W]
                    nc.vector.tensor_mul(p_t[:, k, 0:rc], x_view,
                                         y_t[:, 0:rc, k])
                # k-sum tree, 4 DVE ops: stride-2 slices batch the pair
                # adds of each level into one instruction
                s_t = s_pool.tile([128, 6, RC, W], F16, name="s_t",
                                  tag="s_t")
                with nc.allow_low_precision("fp16 k-sum; tol 2e-2"):
                    a = nc.vector.tensor_add
                    a(s_t[:, 0:4, 0:rc], p_t[:, 0:8:2, 0:rc],
                      p_t[:, 1:8:2, 0:rc])
                    a(s_t[:, 4:6, 0:rc], s_t[:, 0:4:2, 0:rc],
                      s_t[:, 1:4:2, 0:rc])
                    a(s_t[:, 0, 0:rc], s_t[:, 4, 0:rc], s_t[:, 5, 0:rc])
                    a(s_t[:, 1, 0:rc], s_t[:, 0, 0:rc], p_t[:, 8, 0:rc])
                # all o1 writes go through ACT so conv matmuls wait on a
                # single engine (plus the w DMA, covered at startup)
                nc.scalar.copy(o1_sb[:, q0:q0 + rc, 2:W + 2],
                               s_t[:, 1, 0:rc])
                if c == 2:
                    # late wait-merge for the second x piece: DVE is past
                    # chunk-2 work and the DMA finished long ago, so this
                    # costs nothing and keeps chunk-3+ muls at <=2 waits
                    nc.vector.tensor_copy(scr[:, 1:2], x_sb[:, XR - 1, 0:1])

            def conv_chunk(j):
                m0 = CC * j
                jj = j % 2
                ps2 = ps2_pool.tile([128, CC, W], F32, name="ps2", tag="ps2")
                for t in range(9):
                    i3, j3 = divmod(t, 3)
                    for r0 in (0, 2):
                        nc.tensor.matmul(
                            ps2[:, r0:r0 + 2], lhsT=w_sb[:, t],
                            rhs=o1_sb[:, m0 + i3 + r0: m0 + i3 + r0 + 2,
                                      j3 + 1: j3 + 1 + W],
                            start=(t == 0), stop=(t == 8))
                # bias on ACT (PSUM read); leaky combine and the output DMA
                # run once per conv-chunk pair
                if jj == 0:
                    conv_chunk.v_t = v_pool.tile([128, 2, CC, W], F16,
                                                 name="v_t", tag="v_t")
                v_t = conv_chunk.v_t
                nc.scalar.activation(v_t[:, jj], ps2[:], AF.Identity,
                                     bias=b_sb[:, 0:1], scale=1.0)
                if jj == 1:
                    o_t = o_pool.tile([128, 2, CC, W], F16, name="o_t",
                                      tag="o_t")
                    nc.vector.scalar_tensor_tensor(
                        o_t[:], v_t[:], NEG_SLOPE, v_t[:],
                        mybir.AluOpType.mult, mybir.AluOpType.max)
                    nc.sync.dma_start(out_d[:, m0 - CC:m0 + CC], o_t[:])

            reduce_chunk(0)
            for c in range(1, len(RCHUNKS)):
                reduce_chunk(c)
                for j in CONV_AFTER[c]:
                    conv_chunk(j)

    nc.compile()
    return nc


_PROGRAM = None


def _get_program():
    global _PROGRAM
    if _PROGRAM is None:
        _PROGRAM = _build_program()
    return _PROGRAM


def make_in_maps(x, y, fuse_w, fuse_b):
    x16 = np.asarray(x).astype(np.float16)
    y16 = np.asarray(y).astype(np.float16)
    fuse_w = np.asarray(fuse_w, dtype=np.float32)
    fuse_b = np.asarray(fuse_b, dtype=np.float32)

    # block-diagonal conv weights: each partition half (h-half of the
    # slab) contracts with its own copy of W_tap in one K=128 matmul
    wt = np.zeros((128, 9, 128), np.float16)
    for t in range(9):
        i, j = divmod(t, 3)
        wtap = fuse_w[:, :, i, j].T.astype(np.float16)  # [c_in, c_out]
        wt[0:64, t, 0:64] = wtap
        wt[64:128, t, 64:128] = wtap
    bias = np.concatenate([fuse_b, fuse_b]).astype(np.float32)[:, None]

    in_maps = []
    for core in range(NCORES):
        n, hb = divmod(core, 4)
        h0 = hb * HB
        y4 = y16[n].reshape(C, 9, H, W)

        xq = np.zeros((128, XR, XW), np.float16)
        yq = np.zeros((128, Q, 9, W), np.float16)
        for s in (0, 1):
            hs = h0 + HH * s
            # x rows hs-3 .. hs+35 into xq rows 0..38, cols 2..258
            r0, r1 = hs - 3, hs + XR - 3
            c0, c1 = max(r0, 0), min(r1, H)
            xq[64 * s:64 * s + 64, c0 - r0:c1 - r0, D:D + W] = \
                x16[n, :, c0:c1, :]
            # y rows hs-1 .. hs+33 into yq rows 0..34, transposed to
            # [c, q, k, w]
            r0y, r1y = hs - 1, hs + Q - 1
            c0y, c1y = max(r0y, 0), min(r1y, H)
            yq[64 * s:64 * s + 64, c0y - r0y:c1y - r0y] = \
                y4[:, :, c0y:c1y, :].transpose(0, 2, 1, 3)
        in_maps.append({"xq": xq, "yq": yq, "wt": wt, "bias": bias})
    return in_maps


def run(x, y, fuse_w, fuse_b, trace=False, **kw):
    nc = _get_program()
    in_maps = make_in_maps(x, y, fuse_w, fuse_b)
    res = run_bass_kernel_spmd(nc, in_maps, list(range(NCORES)),
                               trace=trace, **kw)
    out = np.empty((N, C, H, W), np.float32)
    for core in range(NCORES):
        n, hb = divmod(core, 4)
        h0 = hb * HB
        o = res.results[core]["out"]
        for s in (0, 1):
            out[n, :, h0 + HH * s:h0 + HH * (s + 1), :] = \
                o[64 * s:64 * s + 64].astype(np.float32)
    return out, res


def kernel(x, y, fuse_w, fuse_b):
    out, _ = run(x, y, fuse_w, fuse_b, trace=False)
    return out


# revision 30
# speedup vs baseline: 1.4628x; 1.1001x over previous
"""DepthDC fused kernel for 8 Trainium2 NeuronCores (fp16 datapath).

Reference computation (N=2, C=64, H=W=256, d=2):
  patches[n,c,k,h,w] = xpad[n,c,h+ki*d, w+kj*d]   (k=3*ki+kj, pad d)
  out1 = sum_k patches * y.reshape(N,C,9,H,W)
  out  = leaky_relu(conv3x3(out1, fuse_w) + fuse_b, 0.2)

Sharding: 8 cores = batch(2) x H-quarters(4). Each core produces a
[64, 64, 256] output slab. Host slices overlapping (haloed, zero-padded)
input slabs per core and converts them to fp16, so no device collectives
are needed and HBM traffic is halved vs fp32 (y dominates at ~19 MB/core).

Per-core layout: the 64 output rows split into two 32-row halves mapped
to SBUF partition halves (partition = c + 64*s). Host pre-packs every
DRAM tensor so each DMA is fully contiguous per partition.

Engine split (PE is utilization-throttled to ~1.2 GHz on this device and
gpsimd streaming poisons shared SBUF bandwidth, so):
  - DVE: the 9 elementwise products per reduce chunk (fp16 2x mode), the
    k-sum as a 4-op merged add tree (stride-2 slices pair 4 adds into
    one instruction), and the final leaky-relu combine
  - PE:  only the 3x3 dense conv (9 taps x 2 row-pair fp16 matmuls,
    PSUM-accumulated)
  - ACT: all o1 writes (keeps conv matmul waits on one engine) and the
    conv bias add from PSUM
Reduce chunks are 6 rows ([6,6,6,6,6,4] over the 34 out1 rows) to
amortize DVE op overheads; conv chunks are 4 rows (PSUM bank pair),
with leaky-relu and the output DMA batched over conv-chunk pairs.
"""

import sys

sys.path.insert(0, "/opt/trn_rl_repo")

import numpy as np

import concourse.bass as bass
import concourse.mybir as mybir
import concourse.tile as tile
from concourse import bacc
from concourse.bass_utils import run_bass_kernel_spmd

F16 = mybir.dt.float16
F32 = mybir.dt.float32
AF = mybir.ActivationFunctionType

N, C, H, W = 2, 64, 256, 256
D = 2  # dilation == pad
NEG_SLOPE = 0.2
NCORES = 8
HB = 64          # output rows per core
HH = 32          # output rows per half
Q = HH + 2       # out1 rows per half (34)
XR = Q + 4       # x rows per half block (38)
XW = W + 2 * D   # padded x width (260)
OW = W + 4       # padded out1 width (260; data at cols 2..258)
RC = 6           # out1 rows per reduce chunk
RCHUNKS = [(0, 6), (6, 6), (12, 6), (18, 6), (24, 6), (30, 4)]
CC = 4           # output rows per conv chunk
# conv chunks to run after each reduce chunk (conv j needs o1 rows
# [4j, 4j+6), available after reduce chunk c when 4j+6 <= end(c));
# FLUSH_AFTER defers each conv-pair's leaky-relu + output DMA until the
# producing matmuls/ACT have certainly finished, so the DVE never stalls
# in-order waiting on PE/ACT
CONV_AFTER = {0: (0,), 1: (1,), 2: (2, 3), 3: (4,), 4: (5, 6), 5: (7,)}
FLUSH_AFTER = {2: (0,), 3: (1,), 4: (2,), 5: (3,)}


def _build_program():
    nc = bacc.Bacc("TRN2", target_bir_lowering=False, debug=False,
                   num_devices=NCORES)

    xq_d = nc.dram_tensor("xq", [128, XR, XW], F16, kind="ExternalInput").ap()
    yq_d = nc.dram_tensor("yq", [128, Q, 9, W], F16,
                          kind="ExternalInput").ap()
    wt_d = nc.dram_tensor("wt", [128, 9, 128], F16, kind="ExternalInput").ap()
    b_d = nc.dram_tensor("bias", [128, 1], F32, kind="ExternalInput").ap()
    out_d = nc.dram_tensor("out", [128, HH, W], F16, kind="ExternalOutput").ap()

    with tile.TileContext(nc) as tc:
        from contextlib import ExitStack
        with ExitStack() as ctx:
            const = ctx.enter_context(tc.tile_pool(name="const", bufs=1))
            y_pool = ctx.enter_context(tc.tile_pool(name="y_pool", bufs=2))
            p_pool = ctx.enter_context(tc.tile_pool(name="p_pool", bufs=2))
            s_pool = ctx.enter_context(tc.tile_pool(name="s_pool", bufs=2))
            o_pool = ctx.enter_context(tc.tile_pool(name="o_pool", bufs=2))
            v_pool = ctx.enter_context(tc.tile_pool(name="v_pool", bufs=2))
            ps2_pool = ctx.enter_context(
                tc.tile_pool(name="ps2_pool", bufs=4, space="PSUM"))

            w_sb = const.tile([128, 9, 128], F16, name="w_sb")
            nc.sync.dma_start(w_sb[:], wt_d)
            b_sb = const.tile([128, 1], F32, name="b_sb")
            nc.sync.dma_start(b_sb[:], b_d)
            # x arrives in three pieces on the ACT engine's DMA queue
            # (parallel with y on the sync queue) so chunk-0 products can
            # start early; rows [0:10) cover chunk 0, [0:22) chunks 1-2,
            # and the rest lands well before chunk 3 reads it.
            x_sb = const.tile([128, XR, XW], F16, name="x_sb")
            nc.scalar.dma_start(x_sb[:, 0:10], xq_d[:, 0:10])
            nc.scalar.dma_start(x_sb[:, 10:22], xq_d[:, 10:22])
            o1_sb = const.tile([128, Q, OW], F16, name="o1_sb")
            # zero the conv W-padding columns once; memset writes without
            # reading (a scale=0 ACT copy reads uninitialized SBUF, and
            # NaN*0 = NaN on a cold device)
            nc.gpsimd.memset(o1_sb[:, :, 1:2], 0.0)
            nc.gpsimd.memset(o1_sb[:, :, OW - 2:OW - 1], 0.0)
            # Wait-merge scratch: one cheap copy per input DMA converts
            # DMA-completion semaphores into engine program order, keeping
            # compute instructions at <=2 foreign wait sems (walrus
            # wait-slot limit). DVE covers x/w; ACT covers b and w (conv
            # matmuls wait on ACT's o1 writes, its Lrelu bias read needs b).
            scr = const.tile([128, 8], F16, name="scr")
            nc.vector.tensor_copy(scr[:, 0:1], x_sb[:, 0, 0:1])
            nc.vector.tensor_copy(scr[:, 2:3], w_sb[:, 0, 0:1])
            scr2 = const.tile([128, 3], F32, name="scr2")
            nc.scalar.activation(scr2[:, 0:1], b_sb[:, 0:1], AF.Copy)
            nc.scalar.activation(scr2[:, 1:2], w_sb[:, 0, 0:1], AF.Copy)
            # fold the Pool memsets into ACT program order so conv matmuls
            # keep a single-engine wait on o1
            nc.scalar.activation(scr2[:, 2:3], o1_sb[:, 0, 1:2], AF.Copy)
            nc.scalar.activation(scr2[:, 2:3], o1_sb[:, 0, OW - 2:OW - 1],
                                 AF.Copy)

            def reduce_chunk(c):
                q0, rc = RCHUNKS[c]
                y_t = y_pool.tile([128, RC, 9, W], F16, name="y_t", tag="y_t")
                nc.sync.dma_start(y_t[:, 0:rc], yq_d[:, q0:q0 + rc])
                if c == 0:
                    nc.scalar.dma_start(x_sb[:, 22:XR], xq_d[:, 22:XR])
                nc.vector.tensor_copy(scr[:, 5:6], y_t[:, 0, 0, 0:1])
                p_t = p_pool.tile([128, 9, RC, W], F16, name="p_t", tag="p_t")
                for k in range(9):
                    ki, kj = divmod(k, 3)
                    x_view = x_sb[:, q0 + 2 * ki: q0 + 2 * ki + rc,
                                  2 * kj: 2 * kj + W]
                    nc.vector.tensor_mul(p_t[:, k, 0:rc], x_view,
                                         y_t[:, 0:rc, k])
                # k-sum tree, 4 DVE ops: stride-2 slices batch the pair
                # adds of each level into one instruction
                s_t = s_pool.tile([128, 6, RC, W], F16, name="s_t",
                                  tag="s_t")
                with nc.allow_low_precision("fp16 k-sum; tol 2e-2"):
                    a = nc.vector.tensor_add
                    a(s_t[:, 0:4, 0:rc], p_t[:, 0:8:2, 0:rc],
                      p_t[:, 1:8:2, 0:rc])
                    a(s_t[:, 4:6, 0:rc], s_t[:, 0:4:2, 0:rc],
                      s_t[:, 1:4:2, 0:rc])
                    a(s_t[:, 0, 0:rc], s_t[:, 4, 0:rc], s_t[:, 5, 0:rc])
                    a(s_t[:, 1, 0:rc], s_t[:, 0, 0:rc], p_t[:, 8, 0:rc])
                # all o1 writes go through ACT so conv matmuls wait on a
                # single engine (plus the w DMA, covered at startup)
                nc.scalar.copy(o1_sb[:, q0:q0 + rc, 2:W + 2],
                               s_t[:, 1, 0:rc])
                if c == 0:
                    # late wait-merges for the later x pieces: DVE is past
                    # the rows they cover, the DMAs are in flight or done,
                    # so these keep later muls at <=2 foreign waits
                    nc.vector.tensor_copy(scr[:, 3:4], x_sb[:, 21, 0:1])
                if c == 1:
                    nc.vector.tensor_copy(scr[:, 1:2], x_sb[:, XR - 1, 0:1])

            v_tiles = {}

            def conv_chunk(j):
                m0 = CC * j
                ps2 = ps2_pool.tile([128, CC, W], F32, name="ps2", tag="ps2")
                for t in range(9):
                    i3, j3 = divmod(t, 3)
                    for r0 in (0, 2):
                        nc.tensor.matmul(
                            ps2[:, r0:r0 + 2], lhsT=w_sb[:, t],
                            rhs=o1_sb[:, m0 + i3 + r0: m0 + i3 + r0 + 2,
                                      j3 + 1: j3 + 1 + W],
                            start=(t == 0), stop=(t == 8))
                # bias on ACT (PSUM read); the pair's leaky combine + DMA
                # happen later in flush_pair
                if j % 2 == 0:
                    v_tiles[j // 2] = v_pool.tile([128, 2, CC, W], F16,
                                                  name="v_t", tag="v_t")
                nc.scalar.activation(v_tiles[j // 2][:, j % 2], ps2[:],
                                     AF.Identity, bias=b_sb[:, 0:1],
                                     scale=1.0)

            def flush_pair(p):
                v_t = v_tiles[p]
                o_t = o_pool.tile([128, 2, CC, W], F16, name="o_t",
                                  tag="o_t")
                nc.vector.scalar_tensor_tensor(
                    o_t[:], v_t[:], NEG_SLOPE, v_t[:],
                    mybir.AluOpType.mult, mybir.AluOpType.max)
                nc.sync.dma_start(out_d[:, 2 * CC * p:2 * CC * (p + 1)],
                                  o_t[:])

            for c in range(len(RCHUNKS)):
                reduce_chunk(c)
                for j in CONV_AFTER.get(c, ()):
                    conv_chunk(j)
                for p in FLUSH_AFTER.get(c, ()):
                    flush_pair(p)

    nc.compile()
    return nc


_PROGRAM = None


def _get_program():
    global _PROGRAM
    if _PROGRAM is None:
        _PROGRAM = _build_program()
    return _PROGRAM


def make_in_maps(x, y, fuse_w, fuse_b):
    x16 = np.asarray(x).astype(np.float16)
    y16 = np.asarray(y).astype(np.float16)
    fuse_w = np.asarray(fuse_w, dtype=np.float32)
    fuse_b = np.asarray(fuse_b, dtype=np.float32)

    # block-diagonal conv weights: each partition half (h-half of the
    # slab) contracts with its own copy of W_tap in one K=128 matmul
    wt = np.zeros((128, 9, 128), np.float16)
    for t in range(9):
        i, j = divmod(t, 3)
        wtap = fuse_w[:, :, i, j].T.astype(np.float16)  # [c_in, c_out]
        wt[0:64, t, 0:64] = wtap
        wt[64:128, t, 64:128] = wtap
    bias = np.concatenate([fuse_b, fuse_b]).astype(np.float32)[:, None]

    in_maps = []
    for core in range(NCORES):
        n, hb = divmod(core, 4)
        h0 = hb * HB
        y4 = y16[n].reshape(C, 9, H, W)

        xq = np.zeros((128, XR, XW), np.float16)
        yq = np.zeros((128, Q, 9, W), np.float16)
        for s in (0, 1):
            hs = h0 + HH * s
            # x rows hs-3 .. hs+35 into xq rows 0..38, cols 2..258
            r0, r1 = hs - 3, hs + XR - 3
            c0, c1 = max(r0, 0), min(r1, H)
            xq[64 * s:64 * s + 64, c0 - r0:c1 - r0, D:D + W] = \
                x16[n, :, c0:c1, :]
            # y rows hs-1 .. hs+33 into yq rows 0..34, transposed to
            # [c, q, k, w]
            r0y, r1y = hs - 1, hs + Q - 1
            c0y, c1y = max(r0y, 0), min(r1y, H)
            yq[64 * s:64 * s + 64, c0y - r0y:c1y - r0y] = \
                y4[:, :, c0y:c1y, :].transpose(0, 2, 1, 3)
        in_maps.append({"xq": xq, "yq": yq, "wt": wt, "bias": bias})
    return in_maps


def run(x, y, fuse_w, fuse_b, trace=False, **kw):
    nc = _get_program()
    in_maps = make_in_maps(x, y, fuse_w, fuse_b)
    res = run_bass_kernel_spmd(nc, in_maps, list(range(NCORES)),
                               trace=trace, **kw)
    out = np.empty((N, C, H, W), np.float32)
    for core in range(NCORES):
        n, hb = divmod(core, 4)
        h0 = hb * HB
        o = res.results[core]["out"]
        for s in (0, 1):
            out[n, :, h0 + HH * s:h0 + HH * (s + 1), :] = \
                o[64 * s:64 * s + 64].astype(np.float32)
    return out, res


def kernel(x, y, fuse_w, fuse_b):
    out, _ = run(x, y, fuse_w, fuse_b, trace=False)
    return out


# revision 31
# speedup vs baseline: 1.6995x; 1.1618x over previous
"""DepthDC fused kernel for 8 Trainium2 NeuronCores (fp16 datapath).

Reference computation (N=2, C=64, H=W=256, d=2):
  patches[n,c,k,h,w] = xpad[n,c,h+ki*d, w+kj*d]   (k=3*ki+kj, pad d)
  out1 = sum_k patches * y.reshape(N,C,9,H,W)
  out  = leaky_relu(conv3x3(out1, fuse_w) + fuse_b, 0.2)

Sharding: 8 cores = batch(2) x H-quarters(4). Each core produces a
[64, 64, 256] output slab. Host slices overlapping (haloed, zero-padded)
input slabs per core and converts them to fp16, so no device collectives
are needed and HBM traffic is halved vs fp32 (y dominates at ~19 MB/core).

Per-core layout: the 64 output rows split into two 32-row halves mapped
to SBUF partition halves (partition = c + 64*s). Host pre-packs every
DRAM tensor so each DMA is fully contiguous per partition.

Engine split (PE is utilization-throttled to ~1.2 GHz here, and gpsimd
streaming poisons the shared SBUF bandwidth, so gpsimd is unused):
  - DVE: 9 elementwise products per 4-row chunk (fp16 2x mode); for most
    chunks the k-sum as a 4-op merged add tree (stride-2 slices pair the
    adds of each level into one instruction); the leaky-relu combine,
    deferred by a chunk so it never stalls the DVE queue
  - PE:  the 3x3 dense conv (9 taps x 2 row-pair fp16 matmuls,
    PSUM-accumulated) plus, for 2 of the 9 chunks, the k-sum via
    identity matmul to offload the DVE
  - ACT: all o1 writes (keeps conv matmul waits single-engine) and the
    conv bias add from PSUM
"""

import sys

sys.path.insert(0, "/opt/trn_rl_repo")

import numpy as np

import concourse.bass as bass
import concourse.mybir as mybir
import concourse.tile as tile
from concourse import bacc
from concourse.bass_utils import run_bass_kernel_spmd

F16 = mybir.dt.float16
F32 = mybir.dt.float32
AF = mybir.ActivationFunctionType

N, C, H, W = 2, 64, 256, 256
D = 2  # dilation == pad
NEG_SLOPE = 0.2
NCORES = 8
HB = 64          # output rows per core
HH = 32          # output rows per half
Q = HH + 2       # out1 rows per half (34)
XR = Q + 4       # x rows per half block (38)
XW = W + 2 * D   # padded x width (260)
OW = W + 4       # padded out1 width (260; data at cols 2..258)
RC = 4           # out1 rows per reduce chunk
RCHUNKS = [(4 * c, 4) for c in range(8)] + [(32, 2)]
CC = 4           # output rows per conv chunk
PE_CHUNKS = (0, 4)   # chunks whose k-sum runs on PE (identity matmul)
# conv j (needs o1 rows [4j, 4j+6)) runs after reduce chunk j+1; each
# conv-pair's leaky-relu + output DMA is deferred (FLUSH_AFTER) so the
# DVE never stalls in-order waiting on PE/ACT
CONV_AFTER = {c: (c - 1,) for c in range(1, 9)}
FLUSH_AFTER = {3: (0,), 5: (1,), 7: (2,), 8: (3,)}


def _build_program():
    nc = bacc.Bacc("TRN2", target_bir_lowering=False, debug=False,
                   num_devices=NCORES)

    xq_d = nc.dram_tensor("xq", [128, XR, XW], F16, kind="ExternalInput").ap()
    yq_d = nc.dram_tensor("yq", [128, Q, 9, W], F16,
                          kind="ExternalInput").ap()
    wt_d = nc.dram_tensor("wt", [128, 9, 128], F16, kind="ExternalInput").ap()
    id_d = nc.dram_tensor("ident", [128, 128], F16, kind="ExternalInput").ap()
    b_d = nc.dram_tensor("bias", [128, 1], F32, kind="ExternalInput").ap()
    out_d = nc.dram_tensor("out", [128, HH, W], F16, kind="ExternalOutput").ap()

    with tile.TileContext(nc) as tc:
        from contextlib import ExitStack
        with ExitStack() as ctx:
            const = ctx.enter_context(tc.tile_pool(name="const", bufs=1))
            y_pool = ctx.enter_context(tc.tile_pool(name="y_pool", bufs=2))
            p_pool = ctx.enter_context(tc.tile_pool(name="p_pool", bufs=2))
            s_pool = ctx.enter_context(tc.tile_pool(name="s_pool", bufs=2))
            o_pool = ctx.enter_context(tc.tile_pool(name="o_pool", bufs=2))
            v_pool = ctx.enter_context(tc.tile_pool(name="v_pool", bufs=2))
            ps1_pool = ctx.enter_context(
                tc.tile_pool(name="ps1_pool", bufs=2, space="PSUM"))
            ps2_pool = ctx.enter_context(
                tc.tile_pool(name="ps2_pool", bufs=2, space="PSUM"))

            w_sb = const.tile([128, 9, 128], F16, name="w_sb")
            nc.sync.dma_start(w_sb[:], wt_d)
            id_sb = const.tile([128, 128], F16, name="id_sb")
            nc.sync.dma_start(id_sb[:], id_d)
            b_sb = const.tile([128, 1], F32, name="b_sb")
            nc.sync.dma_start(b_sb[:], b_d)
            # x arrives in three pieces on the ACT engine's DMA queue
            # (parallel with y on the sync queue) so chunk-0 products can
            # start early; [0:8) covers chunk 0, [0:22) chunks 1-3, the
            # rest lands well before chunk 4 reads it.
            x_sb = const.tile([128, XR, XW], F16, name="x_sb")
            nc.scalar.dma_start(x_sb[:, 0:8], xq_d[:, 0:8])
            nc.scalar.dma_start(x_sb[:, 8:22], xq_d[:, 8:22])
            o1_sb = const.tile([128, Q, OW], F16, name="o1_sb")
            # zero the conv W-padding columns once; memset writes without
            # reading (a scale=0 ACT copy reads uninitialized SBUF, and
            # NaN*0 = NaN on a cold device)
            nc.gpsimd.memset(o1_sb[:, :, 1:2], 0.0)
            nc.gpsimd.memset(o1_sb[:, :, OW - 2:OW - 1], 0.0)
            # Wait-merge scratch: one cheap copy per input DMA converts
            # DMA-completion semaphores into engine program order, keeping
            # compute instructions at <=2 foreign wait sems (walrus
            # wait-slot limit). DVE covers x/w/id; ACT covers b, w and the
            # o1 pad memsets.
            scr = const.tile([128, 8], F16, name="scr")
            nc.vector.tensor_copy(scr[:, 0:1], x_sb[:, 0, 0:1])
            nc.vector.tensor_copy(scr[:, 2:3], w_sb[:, 0, 0:1])
            nc.vector.tensor_copy(scr[:, 4:5], id_sb[:, 0:1])
            scr2 = const.tile([128, 3], F32, name="scr2")
            nc.scalar.activation(scr2[:, 0:1], b_sb[:, 0:1], AF.Copy)
            nc.scalar.activation(scr2[:, 1:2], w_sb[:, 0, 0:1], AF.Copy)
            nc.scalar.activation(scr2[:, 2:3], o1_sb[:, 0, 1:2], AF.Copy)
            nc.scalar.activation(scr2[:, 2:3], o1_sb[:, 0, OW - 2:OW - 1],
                                 AF.Copy)

            def reduce_chunk(c):
                q0, rc = RCHUNKS[c]
                y_t = y_pool.tile([128, RC, 9, W], F16, name="y_t", tag="y_t")
                nc.sync.dma_start(y_t[:, 0:rc], yq_d[:, q0:q0 + rc])
                if c == 0:
                    nc.scalar.dma_start(x_sb[:, 22:XR], xq_d[:, 22:XR])
                nc.vector.tensor_copy(scr[:, 5:6], y_t[:, 0, 0, 0:1])
                p_t = p_pool.tile([128, 9, RC, W], F16, name="p_t", tag="p_t")
                for k in range(9):
                    ki, kj = divmod(k, 3)
                    x_view = x_sb[:, q0 + 2 * ki: q0 + 2 * ki + rc,
                                  2 * kj: 2 * kj + W]
                    nc.vector.tensor_mul(p_t[:, k, 0:rc], x_view,
                                         y_t[:, 0:rc, k])
                if c in PE_CHUNKS:
                    ps1 = ps1_pool.tile([128, RC, W], F32, name="ps1",
                                        tag="ps1")
                    for k in range(9):
                        for r0 in range(0, rc, 2):
                            nc.tensor.matmul(
                                ps1[:, r0:r0 + 2], lhsT=id_sb[:],
                                rhs=p_t[:, k, r0:r0 + 2],
                                start=(k == 0), stop=(k == 8))
                    nc.scalar.copy(o1_sb[:, q0:q0 + rc, 2:W + 2],
                                   ps1[:, 0:rc])
                else:
                    # k-sum tree, 4 DVE ops: stride-2 slices batch the
                    # pair adds of each level into one instruction
                    s_t = s_pool.tile([128, 6, RC, W], F16, name="s_t",
                                      tag="s_t")
                    with nc.allow_low_precision("fp16 k-sum; tol 2e-2"):
                        a = nc.vector.tensor_add
                        a(s_t[:, 0:4, 0:rc], p_t[:, 0:8:2, 0:rc],
                          p_t[:, 1:8:2, 0:rc])
                        a(s_t[:, 4:6, 0:rc], s_t[:, 0:4:2, 0:rc],
                          s_t[:, 1:4:2, 0:rc])
                        a(s_t[:, 0, 0:rc], s_t[:, 4, 0:rc], s_t[:, 5, 0:rc])
                        a(s_t[:, 1, 0:rc], s_t[:, 0, 0:rc], p_t[:, 8, 0:rc])
                    # all o1 writes go through ACT so conv matmuls wait on
                    # a single engine (plus the w DMA, covered at startup)
                    nc.scalar.copy(o1_sb[:, q0:q0 + rc, 2:W + 2],
                                   s_t[:, 1, 0:rc])
                if c == 0:
                    # late wait-merges for the later x pieces: DVE is past
                    # the rows they cover and the DMAs are in flight or
                    # done, so these keep later muls at <=2 foreign waits
                    nc.vector.tensor_copy(scr[:, 3:4], x_sb[:, 21, 0:1])
                if c == 1:
                    nc.vector.tensor_copy(scr[:, 1:2], x_sb[:, XR - 1, 0:1])

            v_tiles = {}

            def conv_chunk(j):
                m0 = CC * j
                ps2 = ps2_pool.tile([128, CC, W], F32, name="ps2", tag="ps2")
                for t in range(9):
                    i3, j3 = divmod(t, 3)
                    for r0 in (0, 2):
                        nc.tensor.matmul(
                            ps2[:, r0:r0 + 2], lhsT=w_sb[:, t],
                            rhs=o1_sb[:, m0 + i3 + r0: m0 + i3 + r0 + 2,
                                      j3 + 1: j3 + 1 + W],
                            start=(t == 0), stop=(t == 8))
                # bias on ACT (PSUM read); the pair's leaky combine + DMA
                # happen later in flush_pair
                if j % 2 == 0:
                    v_tiles[j // 2] = v_pool.tile([128, 2, CC, W], F16,
                                                  name="v_t", tag="v_t")
                nc.scalar.activation(v_tiles[j // 2][:, j % 2], ps2[:],
                                     AF.Identity, bias=b_sb[:, 0:1],
                                     scale=1.0)

            def flush_pair(p):
                v_t = v_tiles[p]
                o_t = o_pool.tile([128, 2, CC, W], F16, name="o_t",
                                  tag="o_t")
                nc.vector.scalar_tensor_tensor(
                    o_t[:], v_t[:], NEG_SLOPE, v_t[:],
                    mybir.AluOpType.mult, mybir.AluOpType.max)
                nc.sync.dma_start(out_d[:, 2 * CC * p:2 * CC * (p + 1)],
                                  o_t[:])

            for c in range(len(RCHUNKS)):
                reduce_chunk(c)
                for j in CONV_AFTER.get(c, ()):
                    conv_chunk(j)
                for p in FLUSH_AFTER.get(c, ()):
                    flush_pair(p)

    nc.compile()
    return nc


_PROGRAM = None


def _get_program():
    global _PROGRAM
    if _PROGRAM is None:
        _PROGRAM = _build_program()
    return _PROGRAM


def make_in_maps(x, y, fuse_w, fuse_b):
    x16 = np.asarray(x).astype(np.float16)
    y16 = np.asarray(y).astype(np.float16)
    fuse_w = np.asarray(fuse_w, dtype=np.float32)
    fuse_b = np.asarray(fuse_b, dtype=np.float32)

    # block-diagonal conv weights: each partition half (h-half of the
    # slab) contracts with its own copy of W_tap in one K=128 matmul
    wt = np.zeros((128, 9, 128), np.float16)
    for t in range(9):
        i, j = divmod(t, 3)
        wtap = fuse_w[:, :, i, j].T.astype(np.float16)  # [c_in, c_out]
        wt[0:64, t, 0:64] = wtap
        wt[64:128, t, 64:128] = wtap
    ident = np.eye(128, dtype=np.float16)
    bias = np.concatenate([fuse_b, fuse_b]).astype(np.float32)[:, None]

    in_maps = []
    for core in range(NCORES):
        n, hb = divmod(core, 4)
        h0 = hb * HB
        y4 = y16[n].reshape(C, 9, H, W)

        xq = np.zeros((128, XR, XW), np.float16)
        yq = np.zeros((128, Q, 9, W), np.float16)
        for s in (0, 1):
            hs = h0 + HH * s
            # x rows hs-3 .. hs+35 into xq rows 0..38, cols 2..258
            r0, r1 = hs - 3, hs + XR - 3
            c0, c1 = max(r0, 0), min(r1, H)
            xq[64 * s:64 * s + 64, c0 - r0:c1 - r0, D:D + W] = \
                x16[n, :, c0:c1, :]
            # y rows hs-1 .. hs+33 into yq rows 0..34, transposed to
            # [c, q, k, w]
            r0y, r1y = hs - 1, hs + Q - 1
            c0y, c1y = max(r0y, 0), min(r1y, H)
            yq[64 * s:64 * s + 64, c0y - r0y:c1y - r0y] = \
                y4[:, :, c0y:c1y, :].transpose(0, 2, 1, 3)
        in_maps.append({"xq": xq, "yq": yq, "wt": wt, "ident": ident,
                        "bias": bias})
    return in_maps


def run(x, y, fuse_w, fuse_b, trace=False, **kw):
    nc = _get_program()
    in_maps = make_in_maps(x, y, fuse_w, fuse_b)
    res = run_bass_kernel_spmd(nc, in_maps, list(range(NCORES)),
                               trace=trace, **kw)
    out = np.empty((N, C, H, W), np.float32)
    for core in range(NCORES):
        n, hb = divmod(core, 4)
        h0 = hb * HB
        o = res.results[core]["out"]
        for s in (0, 1):
            out[n, :, h0 + HH * s:h0 + HH * (s + 1), :] = \
                o[64 * s:64 * s + 64].astype(np.float32)
    return out, res


def kernel(x, y, fuse_w, fuse_b):
    out, _ = run(x, y, fuse_w, fuse_b, trace=False)
    return out


# revision 36
# speedup vs baseline: 1.7306x; 1.0183x over previous
"""DepthDC fused kernel for 8 Trainium2 NeuronCores (fp16 datapath).

Reference computation (N=2, C=64, H=W=256, d=2):
  patches[n,c,k,h,w] = xpad[n,c,h+ki*d, w+kj*d]   (k=3*ki+kj, pad d)
  out1 = sum_k patches * y.reshape(N,C,9,H,W)
  out  = leaky_relu(conv3x3(out1, fuse_w) + fuse_b, 0.2)

Sharding: 8 cores = batch(2) x H-quarters(4). Each core produces a
[64, 64, 256] output slab. Host slices overlapping (haloed, zero-padded)
input slabs per core and converts them to fp16, so no device collectives
are needed and HBM traffic is halved vs fp32 (y dominates at ~19 MB/core).

Per-core layout: the 64 output rows split into two 32-row halves mapped
to SBUF partition halves (partition = c + 64*s). Host pre-packs every
DRAM tensor so each DMA is fully contiguous per partition.

Engine split (PE is utilization-throttled to ~1.2 GHz here, and gpsimd
streaming poisons the shared SBUF bandwidth, so gpsimd is unused):
  - DVE: 9 elementwise products per 4-row chunk (fp16 2x mode); for most
    chunks the k-sum as a 4-op merged add tree (stride-2 slices pair the
    adds of each level into one instruction); the leaky-relu combine,
    deferred by a chunk so it never stalls the DVE queue
  - PE:  the 3x3 dense conv (9 taps x 2 row-pair fp16 matmuls,
    PSUM-accumulated) plus, for 2 of the 9 chunks, the k-sum via
    identity matmul to offload the DVE
  - ACT: all o1 writes (keeps conv matmul waits single-engine) and the
    conv bias add from PSUM
"""

import sys

sys.path.insert(0, "/opt/trn_rl_repo")

import numpy as np

import concourse.bass as bass
import concourse.mybir as mybir
import concourse.tile as tile
from concourse import bacc
from concourse.bass_utils import run_bass_kernel_spmd

F16 = mybir.dt.float16
F32 = mybir.dt.float32
AF = mybir.ActivationFunctionType

N, C, H, W = 2, 64, 256, 256
D = 2  # dilation == pad
NEG_SLOPE = 0.2
NCORES = 8
HB = 64          # output rows per core
HH = 32          # output rows per half
Q = HH + 2       # out1 rows per half (34)
XR = Q + 4       # x rows per half block (38)
XW = W + 2 * D   # padded x width (260)
OW = W + 4       # padded out1 width (260; data at cols 2..258)
RC = 4           # out1 rows per reduce chunk
RCHUNKS = [(4 * c, 4) for c in range(8)] + [(32, 2)]
CC = 4           # output rows per conv chunk
PE_CHUNKS = (0, 4)   # chunks whose k-sum runs on PE (identity matmul)
# conv j (needs o1 rows [4j, 4j+6)) runs after reduce chunk j+1; each
# conv-pair's leaky-relu + output DMA is deferred (FLUSH_AFTER) so the
# DVE never stalls in-order waiting on PE/ACT
CONV_AFTER = {c: (c - 1,) for c in range(1, 9)}
FLUSH_AFTER = {3: (0,), 5: (1,), 7: (2,)}


def _build_program():
    nc = bacc.Bacc("TRN2", target_bir_lowering=False, debug=False,
                   num_devices=NCORES)

    xq_d = nc.dram_tensor("xq", [128, XR, XW], F16, kind="ExternalInput").ap()
    yq_d = nc.dram_tensor("yq", [128, Q, 9, W], F16,
                          kind="ExternalInput").ap()
    wt_d = nc.dram_tensor("wt", [128, 9, 128], F16, kind="ExternalInput").ap()
    id_d = nc.dram_tensor("ident", [128, 128], F16, kind="ExternalInput").ap()
    b_d = nc.dram_tensor("bias", [128, 1], F32, kind="ExternalInput").ap()
    out_d = nc.dram_tensor("out", [128, HH, W], F16, kind="ExternalOutput").ap()

    with tile.TileContext(nc) as tc:
        from contextlib import ExitStack
        with ExitStack() as ctx:
            const = ctx.enter_context(tc.tile_pool(name="const", bufs=1))
            y_pool = ctx.enter_context(tc.tile_pool(name="y_pool", bufs=2))
            p_pool = ctx.enter_context(tc.tile_pool(name="p_pool", bufs=2))
            s_pool = ctx.enter_context(tc.tile_pool(name="s_pool", bufs=2))
            o_pool = ctx.enter_context(tc.tile_pool(name="o_pool", bufs=2))
            v_pool = ctx.enter_context(tc.tile_pool(name="v_pool", bufs=2))
            ps1_pool = ctx.enter_context(
                tc.tile_pool(name="ps1_pool", bufs=2, space="PSUM"))
            ps2_pool = ctx.enter_context(
                tc.tile_pool(name="ps2_pool", bufs=2, space="PSUM"))

            # chunk 0's y slab is the critical path to the first products:
            # issue its DMA before every other transfer on the sync queue
            y0_t = y_pool.tile([128, RC, 9, W], F16, name="y_t", tag="y_t")
            nc.sync.dma_start(y0_t[:, 0:RCHUNKS[0][1]],
                              yq_d[:, 0:RCHUNKS[0][1]])
            w_sb = const.tile([128, 9, 128], F16, name="w_sb")
            nc.sync.dma_start(w_sb[:], wt_d)
            id_sb = const.tile([128, 128], F16, name="id_sb")
            nc.sync.dma_start(id_sb[:], id_d)
            b_sb = const.tile([128, 1], F32, name="b_sb")
            nc.sync.dma_start(b_sb[:], b_d)
            # x arrives in three pieces on the ACT engine's DMA queue
            # (parallel with y on the sync queue) so chunk-0 products can
            # start early; [0:8) covers chunk 0, [0:22) chunks 1-3, the
            # rest lands well before chunk 4 reads it.
            x_sb = const.tile([128, XR, XW], F16, name="x_sb")
            nc.scalar.dma_start(x_sb[:, 0:8], xq_d[:, 0:8])
            nc.scalar.dma_start(x_sb[:, 8:22], xq_d[:, 8:22])
            o1_sb = const.tile([128, Q, OW], F16, name="o1_sb")
            # zero the conv W-padding columns once; memset writes without
            # reading (a scale=0 ACT copy reads uninitialized SBUF, and
            # NaN*0 = NaN on a cold device)
            nc.gpsimd.memset(o1_sb[:, :, 1:2], 0.0)
            nc.gpsimd.memset(o1_sb[:, :, OW - 2:OW - 1], 0.0)
            # Wait-merge scratch: one cheap copy per input DMA converts
            # DMA-completion semaphores into engine program order, keeping
            # compute instructions at <=2 foreign wait sems (walrus
            # wait-slot limit). DVE covers x/w/id; ACT covers b, w and the
            # o1 pad memsets.
            scr = const.tile([128, 8], F16, name="scr")
            nc.vector.tensor_copy(scr[:, 0:1], x_sb[:, 0, 0:1])
            nc.vector.tensor_copy(scr[:, 2:3], w_sb[:, 0, 0:1])
            nc.vector.tensor_copy(scr[:, 4:5], id_sb[:, 0:1])
            scr2 = const.tile([128, 3], F32, name="scr2")
            nc.scalar.activation(scr2[:, 0:1], b_sb[:, 0:1], AF.Copy)
            nc.scalar.activation(scr2[:, 1:2], w_sb[:, 0, 0:1], AF.Copy)
            nc.scalar.activation(scr2[:, 2:3], o1_sb[:, 0, 1:2], AF.Copy)
            nc.scalar.activation(scr2[:, 2:3], o1_sb[:, 0, OW - 2:OW - 1],
                                 AF.Copy)

            def reduce_chunk(c):
                q0, rc = RCHUNKS[c]
                if c == 0:
                    y_t = y0_t
                else:
                    y_t = y_pool.tile([128, RC, 9, W], F16, name="y_t",
                                      tag="y_t")
                    nc.sync.dma_start(y_t[:, 0:rc], yq_d[:, q0:q0 + rc])
                if c == 2:
                    # third x piece: first needed by chunk 4, issued here
                    # so it never delays the startup-critical transfers
                    nc.scalar.dma_start(x_sb[:, 22:XR], xq_d[:, 22:XR])
                nc.vector.tensor_copy(scr[:, 5:6], y_t[:, 0, 0, 0:1])
                p_t = p_pool.tile([128, 9, RC, W], F16, name="p_t", tag="p_t")
                for k in range(9):
                    ki, kj = divmod(k, 3)
                    x_view = x_sb[:, q0 + 2 * ki: q0 + 2 * ki + rc,
                                  2 * kj: 2 * kj + W]
                    nc.vector.tensor_mul(p_t[:, k, 0:rc], x_view,
                                         y_t[:, 0:rc, k])
                if c in PE_CHUNKS:
                    ps1 = ps1_pool.tile([128, RC, W], F32, name="ps1",
                                        tag="ps1")
                    for k in range(9):
                        for r0 in range(0, rc, 2):
                            nc.tensor.matmul(
                                ps1[:, r0:r0 + 2], lhsT=id_sb[:],
                                rhs=p_t[:, k, r0:r0 + 2],
                                start=(k == 0), stop=(k == 8))
                    nc.scalar.copy(o1_sb[:, q0:q0 + rc, 2:W + 2],
                                   ps1[:, 0:rc])
                else:
                    # k-sum tree, 4 DVE ops: stride-2 slices batch the
                    # pair adds of each level into one instruction
                    s_t = s_pool.tile([128, 6, RC, W], F16, name="s_t",
                                      tag="s_t")
                    with nc.allow_low_precision("fp16 k-sum; tol 2e-2"):
                        a = nc.vector.tensor_add
                        a(s_t[:, 0:4, 0:rc], p_t[:, 0:8:2, 0:rc],
                          p_t[:, 1:8:2, 0:rc])
                        a(s_t[:, 4:6, 0:rc], s_t[:, 0:4:2, 0:rc],
                          s_t[:, 1:4:2, 0:rc])
                        a(s_t[:, 0, 0:rc], s_t[:, 4, 0:rc], s_t[:, 5, 0:rc])
                        a(s_t[:, 1, 0:rc], s_t[:, 0, 0:rc], p_t[:, 8, 0:rc])
                    # all o1 writes go through ACT so conv matmuls wait on
                    # a single engine (plus the w DMA, covered at startup)
                    nc.scalar.copy(o1_sb[:, q0:q0 + rc, 2:W + 2],
                                   s_t[:, 1, 0:rc])
                if c == 0:
                    # late wait-merges for the later x pieces: DVE is past
                    # the rows they cover and the DMAs are in flight or
                    # done, so these keep later muls at <=2 foreign waits
                    nc.vector.tensor_copy(scr[:, 3:4], x_sb[:, 21, 0:1])
                if c == 3:
                    nc.vector.tensor_copy(scr[:, 1:2], x_sb[:, XR - 1, 0:1])

            v_tiles = {}

            def conv_chunk(j):
                m0 = CC * j
                ps2 = ps2_pool.tile([128, CC, W], F32, name="ps2", tag="ps2")
                for t in range(9):
                    i3, j3 = divmod(t, 3)
                    for r0 in (0, 2):
                        nc.tensor.matmul(
                            ps2[:, r0:r0 + 2], lhsT=w_sb[:, t],
                            rhs=o1_sb[:, m0 + i3 + r0: m0 + i3 + r0 + 2,
                                      j3 + 1: j3 + 1 + W],
                            start=(t == 0), stop=(t == 8))
                # bias on ACT (PSUM read); the pair's leaky combine + DMA
                # happen later in flush_pair
                if j % 2 == 0:
                    v_tiles[j // 2] = v_pool.tile([128, 2, CC, W], F16,
                                                  name="v_t", tag="v_t")
                nc.scalar.activation(v_tiles[j // 2][:, j % 2], ps2[:],
                                     AF.Identity, bias=b_sb[:, 0:1],
                                     scale=1.0)

            def flush_pair(p):
                v_t = v_tiles[p]
                o_t = o_pool.tile([128, 2, CC, W], F16, name="o_t",
                                  tag="o_t")
                nc.vector.scalar_tensor_tensor(
                    o_t[:], v_t[:], NEG_SLOPE, v_t[:],
                    mybir.AluOpType.mult, mybir.AluOpType.max)
                nc.sync.dma_start(out_d[:, 2 * CC * p:2 * CC * (p + 1)],
                                  o_t[:])

            def flush_half(p, h):
                # tail optimization: flush the last pair one conv chunk at
                # a time so only the final 4 rows wait on the last conv
                v_t = v_tiles[p]
                o_t = o_pool.tile([128, CC, W], F16, name="o_h",
                                  tag="o_h")
                nc.vector.scalar_tensor_tensor(
                    o_t[:], v_t[:, h], NEG_SLOPE, v_t[:, h],
                    mybir.AluOpType.mult, mybir.AluOpType.max)
                m0 = CC * (2 * p + h)
                nc.sync.dma_start(out_d[:, m0:m0 + CC], o_t[:])

            for c in range(len(RCHUNKS)):
                reduce_chunk(c)
                if c == 8:
                    # conv 6's bias result is ready; drain it before the
                    # final conv so only conv 7's rows pay the tail stall
                    flush_half(3, 0)
                for j in CONV_AFTER.get(c, ()):
                    conv_chunk(j)
                for p in FLUSH_AFTER.get(c, ()):
                    flush_pair(p)
            flush_half(3, 1)

    nc.compile()
    return nc


_PROGRAM = None


def _get_program():
    global _PROGRAM
    if _PROGRAM is None:
        _PROGRAM = _build_program()
    return _PROGRAM


def make_in_maps(x, y, fuse_w, fuse_b):
    x16 = np.asarray(x).astype(np.float16)
    y16 = np.asarray(y).astype(np.float16)
    fuse_w = np.asarray(fuse_w, dtype=np.float32)
    fuse_b = np.asarray(fuse_b, dtype=np.float32)

    # block-diagonal conv weights: each partition half (h-half of the
    # slab) contracts with its own copy of W_tap in one K=128 matmul
    wt = np.zeros((128, 9, 128), np.float16)
    for t in range(9):
        i, j = divmod(t, 3)
        wtap = fuse_w[:, :, i, j].T.astype(np.float16)  # [c_in, c_out]
        wt[0:64, t, 0:64] = wtap
        wt[64:128, t, 64:128] = wtap
    ident = np.eye(128, dtype=np.float16)
    bias = np.concatenate([fuse_b, fuse_b]).astype(np.float32)[:, None]

    in_maps = []
    for core in range(NCORES):
        n, hb = divmod(core, 4)
        h0 = hb * HB
        y4 = y16[n].reshape(C, 9, H, W)

        xq = np.zeros((128, XR, XW), np.float16)
        yq = np.zeros((128, Q, 9, W), np.float16)
        for s in (0, 1):
            hs = h0 + HH * s
            # x rows hs-3 .. hs+35 into xq rows 0..38, cols 2..258
            r0, r1 = hs - 3, hs + XR - 3
            c0, c1 = max(r0, 0), min(r1, H)
            xq[64 * s:64 * s + 64, c0 - r0:c1 - r0, D:D + W] = \
                x16[n, :, c0:c1, :]
            # y rows hs-1 .. hs+33 into yq rows 0..34, transposed to
            # [c, q, k, w]
            r0y, r1y = hs - 1, hs + Q - 1
            c0y, c1y = max(r0y, 0), min(r1y, H)
            yq[64 * s:64 * s + 64, c0y - r0y:c1y - r0y] = \
                y4[:, :, c0y:c1y, :].transpose(0, 2, 1, 3)
        in_maps.append({"xq": xq, "yq": yq, "wt": wt, "ident": ident,
                        "bias": bias})
    return in_maps


def run(x, y, fuse_w, fuse_b, trace=False, **kw):
    nc = _get_program()
    in_maps = make_in_maps(x, y, fuse_w, fuse_b)
    res = run_bass_kernel_spmd(nc, in_maps, list(range(NCORES)),
                               trace=trace, **kw)
    out = np.empty((N, C, H, W), np.float32)
    for core in range(NCORES):
        n, hb = divmod(core, 4)
        h0 = hb * HB
        o = res.results[core]["out"]
        for s in (0, 1):
            out[n, :, h0 + HH * s:h0 + HH * (s + 1), :] = \
                o[64 * s:64 * s + 64].astype(np.float32)
    return out, res


def kernel(x, y, fuse_w, fuse_b):
    out, _ = run(x, y, fuse_w, fuse_b, trace=False)
    return out
